# revision 1
# baseline (speedup 1.0000x reference)
"""Trainium2 Bass kernel for nn_Attention_17454747091547.

Segmented-projection 2-head attention over seq=16, head_dim=3, batch 262144.
Pure data parallel across 8 NeuronCores (32768 batch elements per core).

Host-side precompute folds the per-position segment weights into 96x96
block-diagonal projection matrices, so QKV + out-proj become single TensorE
matmuls. x arrives host-pre-transposed as [96, 32768] bf16 and is preloaded
into SBUF in chunks; output is accumulated transposed in SBUF (bf16) and
written out with a few large DMAs. This keeps every PE/DMA instruction at
<=1 sync-wait (walrus codegen limit): all PE operands except the x chunks
are produced by DVE, and a setup dummy matmul makes PE observe the DVE
semaphore before tile 0.

The attention core (scores/softmax/attn@V) runs on VectorE/ScalarE with
batch on partitions.
"""

import numpy as np
import ml_dtypes

import concourse.bass as bass
import concourse.tile as tile
from concourse import bacc
from concourse import mybir
from concourse.bass_utils import run_bass_kernel_spmd

SEG = [0, 1, 1, 1, 1, 1, 2, 2, 2, 3, 4, 4, 4, 4, 4, 4]
N_CORES = 8
B_TOTAL = 262144
B_SHARD = B_TOTAL // N_CORES  # 32768
P = 128
NTILES = B_SHARD // P  # 256
TILES_PER_CHUNK = 8
NCHUNKS = NTILES // TILES_PER_CHUNK  # 32
CHUNK_COLS = TILES_PER_CHUNK * P  # 1024
S = 16
D = 6
H = 2
HD = 3
F = S * D  # 96

_nc_cache = {}


def _build_weight(W, scale=1.0):
    """[5,6,6] -> [96,96] f32 blockdiag of W[seg[s]].T."""
    Wa = np.zeros((F, F), np.float32)
    for s in range(S):
        Wa[s * D:(s + 1) * D, s * D:(s + 1) * D] = W[SEG[s]].T * scale
    return Wa


def _build_graph():
    nc = bacc.Bacc()
    f32 = mybir.dt.float32
    bf16 = mybir.dt.bfloat16

    xt_ext = nc.declare_dram_parameter("xt", [F + 1, B_SHARD], bf16, isOutput=False)
    w_exts = {}
    b_exts = {}
    for nm in ["wq", "wk", "wv"]:
        w_exts[nm] = nc.declare_dram_parameter(nm, [F + 1, F], bf16, isOutput=False)
    w_exts["wo"] = nc.declare_dram_parameter("wo", [F, F], bf16, isOutput=False)
    bo_ext = nc.declare_dram_parameter("bo", [F], f32, isOutput=False)
    id_ext = nc.declare_dram_parameter("ident", [P, P], bf16, isOutput=False)
    out_ext = nc.declare_dram_parameter("out", [F, B_SHARD], bf16, isOutput=True)

    mult = mybir.AluOpType.mult
    add = mybir.AluOpType.add

    with tile.TileContext(nc) as tc:
        with (
            tc.tile_pool(name="const", bufs=1) as const,
            tc.tile_pool(name="sbqkv", bufs=6) as sbqkv,
            tc.tile_pool(name="sbwork", bufs=6) as sbwork,
            tc.tile_pool(name="gpwork", bufs=6) as gpwork,
            tc.tile_pool(name="sbctxT", bufs=3) as sbctxT,
            tc.tile_pool(name="psT", bufs=2, space="PSUM") as psT,
            tc.tile_pool(name="psQKV", bufs=4, space="PSUM") as psQKV,
            tc.tile_pool(name="psO", bufs=2, space="PSUM") as psO,
        ):
            # --- setup: stage every PE-read constant through DVE ---
            w_dma = {}
            w_sb = {}
            for nm in ["wq", "wk", "wv", "wo"]:
                rows = F + 1 if nm != "wo" else F
                w_dma[nm] = const.tile([rows, F], bf16, tag=nm + "d",
                                       name="wd_" + nm)
                nc.sync.dma_start(out=w_dma[nm], in_=w_exts[nm][:])
                w_sb[nm] = const.tile([rows, F], bf16, tag=nm, name="w_" + nm)
                nc.vector.tensor_copy(w_sb[nm][:], w_dma[nm][:])
            id_dma = const.tile([P, P], bf16)
            nc.sync.dma_start(out=id_dma, in_=id_ext[:])
            I128b = const.tile([P, P], bf16)
            nc.vector.tensor_copy(I128b[:], id_dma[:])
            bo_dma = const.tile([F, 1], f32)
            nc.sync.dma_start(out=bo_dma, in_=bo_ext[:].unsqueeze(1))
            bo_sb = const.tile([F, 1], f32)
            nc.vector.tensor_copy(bo_sb[:], bo_dma[:])

            # dummy exp: loads the exp_and_others ACT table set during setup
            # instead of stalling tile-0's first softmax
            act_warm = const.tile([1, 1], f32)
            nc.scalar.activation(act_warm, bo_sb[0:1, 0:1],
                                 mybir.ActivationFunctionType.Exp)

            # dummy matmul: PE observes the DVE setup tick, so tile-0 PE
            # instructions carry at most one sync-wait (walrus limit)
            psDummy = psT.tile([1, 1], f32, tag="t")
            nc.tensor.matmul(psDummy, lhsT=I128b[0:1, 0:1],
                             rhs=I128b[0:1, 0:1], start=True, stop=True)

            # x chunks: separate tiles so chunk DMAs have no mutual deps.
            # Leading chunks are finer so tile-0 compute starts sooner.
            chunk_tiles = [1, 1, 2, 4] + [8] * ((NTILES - 8) // 8)
            assert sum(chunk_tiles) == NTILES
            xchunk_of_tile = []
            xcol_of_tile = []
            xc = []
            t0 = 0
            for c, nt in enumerate(chunk_tiles):
                xtile = const.tile([F + 1, nt * P], bf16, tag=f"xc{c}",
                                   name=f"xc_{c}")
                nc.sync.dma_start(
                    out=xtile,
                    in_=xt_ext[:, t0 * P:(t0 + nt) * P])
                xc.append(xtile)
                for j in range(nt):
                    xchunk_of_tile.append(c)
                    xcol_of_tile.append(j * P)
                t0 += nt

            # transposed output accumulators; trailing chunks are finer so
            # the final drain DMA is small
            out_chunk_tiles = [32] * 7 + [16, 8, 4, 2, 1, 1]
            assert sum(out_chunk_tiles) == NTILES
            outc = []
            ochunk_of_tile = []
            ocol_of_tile = []
            ostart_col = []
            t0 = 0
            for c, nt in enumerate(out_chunk_tiles):
                otile = const.tile([F, nt * P], bf16, tag=f"oc{c}",
                                   name=f"outc_{c}")
                outc.append(otile)
                ostart_col.append(t0 * P)
                for j in range(nt):
                    ochunk_of_tile.append(c)
                    ocol_of_tile.append(j * P)
                t0 += nt

            for it in range(NTILES):
                xslice = xc[xchunk_of_tile[it]][
                    :, xcol_of_tile[it]:xcol_of_tile[it] + P]

                # Q/K/V in batch-on-partition layout [128, (s,h,d)]
                psQ = psQKV.tile([P, F], f32, tag="qkv")
                psK = psQKV.tile([P, F], f32, tag="qkv")
                psV = psQKV.tile([P, F], f32, tag="qkv")
                nc.tensor.matmul(psQ, lhsT=xslice, rhs=w_sb["wq"][:],
                                 start=True, stop=True)
                nc.tensor.matmul(psK, lhsT=xslice, rhs=w_sb["wk"][:],
                                 start=True, stop=True)
                nc.tensor.matmul(psV, lhsT=xslice, rhs=w_sb["wv"][:],
                                 start=True, stop=True)

                q_sb = sbqkv.tile([P, S, H, HD], bf16, tag="q")
                k_sb = sbqkv.tile([P, S, H, HD], bf16, tag="k")
                v_sb = sbqkv.tile([P, H, HD, S], bf16, tag="v")
                for dst, ps in ((q_sb, psQ), (k_sb, psK), (v_sb, psV)):
                    nc.scalar.activation(
                        dst[:].rearrange("p a b c -> p (a b c)"), ps[:],
                        mybir.ActivationFunctionType.Copy,
                    )

                # scores products + reduce over d
                qv = q_sb[:].transpose([0, 2, 1, 3])  # [p, h, q, d]
                kv = k_sb[:].transpose([0, 2, 1, 3])  # [p, h, k, d]
                prod = gpwork.tile([P, H, S, S, HD], bf16, tag="prod")
                scores = sbwork.tile([P, H, S, S], f32, tag="scores")
                for h in range(H):
                    nc.gpsimd.tensor_mul(
                        prod[:, h],
                        qv[:, h].unsqueeze(2).broadcast_to([P, S, S, HD]),
                        kv[:, h].unsqueeze(1).broadcast_to([P, S, S, HD]),
                    )
                    if h == 0:
                        # d-sum for head 0 on gpsimd (two chained adds)
                        nc.gpsimd.tensor_add(
                            prod[:, h, :, :, 0], prod[:, h, :, :, 0],
                            prod[:, h, :, :, 1],
                        )
                        nc.gpsimd.tensor_add(
                            scores[:, h], prod[:, h, :, :, 0],
                            prod[:, h, :, :, 2],
                        )
                    else:
                        nc.vector.tensor_reduce(
                            scores[:, h], prod[:, h],
                            axis=mybir.AxisListType.X, op=add,
                        )

                # softmax (scores bounded; no max subtraction needed)
                E = sbwork.tile([P, H, S, S], bf16, tag="E")
                nc.scalar.activation(E[:], scores[:],
                                     mybir.ActivationFunctionType.Exp)
                denom = sbwork.tile([P, H, S], f32, tag="denom")
                nc.vector.tensor_reduce(
                    denom[:].rearrange("p h q -> p (h q)"),
                    E[:].rearrange("p h q k -> p (h q) k"),
                    axis=mybir.AxisListType.X, op=add,
                )
                rden = sbwork.tile([P, H, S], f32, tag="rden")
                nc.vector.reciprocal(rden[:], denom[:])

                # ctx_unnorm[h,q,d] = sum_k E[h,q,k] * V[h,d,k]
                # layout keeps innermost k stride-1 on all APs -> DVE 2x mode
                prod2 = sbwork.tile([P, H, S, HD, S], bf16, tag="prod2")
                ctxu = sbwork.tile([P, H, S, HD], f32, tag="ctxu")
                for h in (0, 1):
                    nc.vector.tensor_mul(
                        prod2[:, h],
                        E[:, h].unsqueeze(2).broadcast_to([P, S, HD, S]),
                        v_sb[:, h].unsqueeze(1).broadcast_to([P, S, HD, S]),
                    )
                    nc.vector.tensor_reduce(
                        ctxu[:, h], prod2[:, h],
                        axis=mybir.AxisListType.X, op=add,
                    )

                # normalize; write in (s, h, d) order to match out-proj layout
                ctx_b = sbqkv.tile([P, S, H, HD], bf16, tag="ctx")
                nc.vector.tensor_mul(
                    ctx_b[:].transpose([0, 2, 1, 3]),
                    ctxu[:],
                    rden[:].unsqueeze(3).broadcast_to([P, H, S, HD]),
                )

                # out-proj (transposed): outT_tile = Wo_blk.T @ ctx^T + bo
                psC = psT.tile([F, P], bf16, tag="t")
                nc.tensor.transpose(
                    psC, ctx_b[:].rearrange("p s h d -> p (s h d)"), I128b[:]
                )
                ctxT = sbctxT.tile([F, P], bf16)
                nc.scalar.activation(ctxT[:], psC[:],
                                     mybir.ActivationFunctionType.Copy)
                psOutT = psO.tile([F, P], f32)
                nc.tensor.matmul(psOutT, lhsT=w_sb["wo"][:], rhs=ctxT[:],
                                 start=True, stop=True)
                oc_idx = ochunk_of_tile[it]
                oc_off = ocol_of_tile[it]
                nc.scalar.activation(
                    outc[oc_idx][:, oc_off:oc_off + P], psOutT[:],
                    mybir.ActivationFunctionType.Identity,
                    bias=bo_sb[:], scale=1.0,
                )

            # final output DMAs, one per chunk
            for c, nt in enumerate(out_chunk_tiles):
                nc.sync.dma_start(
                    out=out_ext[:, ostart_col[c]:ostart_col[c] + nt * P],
                    in_=outc[c][:])

    return nc


def get_graph():
    if "nc" not in _nc_cache:
        nc = _build_graph()
        nc.finalize()
        _nc_cache["nc"] = nc
    return _nc_cache["nc"]


def _aug(Wblk, bvec):
    """[96,96] weight + [96] bias -> [97,96] with bias row."""
    return np.concatenate([Wblk, bvec[None, :]], axis=0)


def prepare_in_maps(x, Wq, bq, Wk, bk, Wv, bv, Wo, bo):
    bf16 = ml_dtypes.bfloat16
    sc = 1.0 / np.sqrt(np.float32(HD))
    seg = np.asarray(SEG)
    bqf = (bq[seg].reshape(F) * sc).astype(np.float32)
    bkf = bk[seg].reshape(F).astype(np.float32)
    bvf = bv[seg].reshape(F).astype(np.float32)
    wq = _aug(_build_weight(Wq, sc), bqf).astype(bf16)
    wk = _aug(_build_weight(Wk), bkf).astype(bf16)
    # V projection columns permuted to (h, d, k) order
    perm = np.empty(F, np.int64)
    for h in range(H):
        for d in range(HD):
            for k in range(S):
                perm[h * HD * S + d * S + k] = k * D + h * HD + d
    wv = _aug(_build_weight(Wv), bvf)[:, perm].astype(bf16)
    wo_full = np.zeros((F, F), np.float32)
    for s in range(S):
        wo_full[s * D:(s + 1) * D, s * D:(s + 1) * D] = Wo.T
    wo = wo_full.astype(bf16)
    bof = np.tile(bo, S).astype(np.float32)
    ident = np.eye(P, dtype=bf16)

    xf = np.asarray(x, np.float32).reshape(B_TOTAL, F).astype(bf16)
    ones = np.ones((1, B_SHARD), dtype=bf16)
    in_maps = []
    for c in range(N_CORES):
        shard = np.concatenate([np.ascontiguousarray(
            xf[c * B_SHARD:(c + 1) * B_SHARD].T), ones], axis=0)  # [97, B]
        in_maps.append({"xt": shard, "wq": wq, "wk": wk, "wv": wv, "wo": wo,
                        "bo": bof, "ident": ident})
    return in_maps


def kernel(x, Wq, bq, Wk, bk, Wv, bv, Wo, bo):
    nc = get_graph()
    in_maps = prepare_in_maps(x, Wq, bq, Wk, bk, Wv, bv, Wo, bo)
    res = run_bass_kernel_spmd(nc, in_maps, core_ids=list(range(N_CORES)))
    outs = [np.asarray(res.results[c]["out"]).astype(np.float32).T
            for c in range(N_CORES)]  # each [32768, 96]
    out = np.concatenate(outs, axis=0)
    return np.ascontiguousarray(out.reshape(B_TOTAL, S, D))



# revision 2
# speedup vs baseline: 1.0088x; 1.0088x over previous
"""Trainium2 Bass kernel for nn_Attention_17454747091547 — v6.

Measured-HW-cost driven redesign. Per-instruction DVE overhead is ~500ns
on hardware, so everything fusable is fused across W=4 batch sub-tiles
(512 batch rows per macro-iteration); only the broadcast products
(QK, AV) stay per-(subtile, head) — their APs hit the 3-free-dim ISA
limit otherwise.

Per macro-iter (4 tiles, 512 batch):
  PE  : 4 fused QKV projections (one [97,288] matmul each into a
        bank-aligned PSUM slot), 4 ctx transposes, 4 out-projections
  ACT : 1 fused QKV PSUM->SBUF copy, 1 fused exp, 4 ctxT copies,
        4 bias+store copies
  DVE : 8 QK product muls, 1 denom tensor_reduce over E, 1 reciprocal,
        8 AV product muls, tree steps t1+t2, 1 fused normalize
  Pool: d-reduce (2 fused strided adds), tree steps t3+t4

x and output stream through small rotating SBUF buffers (one DMA per
macro-iter each way); weights/identity stay resident.
"""

import numpy as np
import ml_dtypes

import concourse.bass as bass
import concourse.tile as tile
from concourse import bacc
from concourse import mybir
from concourse.bass_utils import run_bass_kernel_spmd

SEG = [0, 1, 1, 1, 1, 1, 2, 2, 2, 3, 4, 4, 4, 4, 4, 4]
N_CORES = 8
B_TOTAL = 262144
B_SHARD = B_TOTAL // N_CORES  # 32768
P = 128
W = 4                      # sub-tiles per macro-iteration
NTILES = B_SHARD // P      # 256
NMACRO = NTILES // W       # 64
S = 16
D = 6
H = 2
HD = 3
F = S * D                  # 96
QKV_COLS = 3 * F           # 288 = q(96,(s,h,d)) | k(96,(s,h,d)) | v(96,(h,d,k))

_nc_cache = {}


def _build_weight(W_, scale=1.0):
    """[5,6,6] -> [96,96] f32 blockdiag of W[seg[s]].T (cols (s,o))."""
    Wa = np.zeros((F, F), np.float32)
    for s in range(S):
        Wa[s * D:(s + 1) * D, s * D:(s + 1) * D] = W_[SEG[s]].T * scale
    return Wa


def _build_graph():
    nc = bacc.Bacc()
    f32 = mybir.dt.float32
    bf16 = mybir.dt.bfloat16
    add = mybir.AluOpType.add

    xt_ext = nc.declare_dram_parameter("xt", [F + 1, B_SHARD], bf16,
                                       isOutput=False)
    wqkv_ext = nc.declare_dram_parameter("wqkv", [F + 1, QKV_COLS], bf16,
                                         isOutput=False)
    wo_ext = nc.declare_dram_parameter("wo", [F, F], bf16, isOutput=False)
    bo_ext = nc.declare_dram_parameter("bo", [F], f32, isOutput=False)
    id_ext = nc.declare_dram_parameter("ident", [P, P], bf16, isOutput=False)
    out_ext = nc.declare_dram_parameter("out", [F, B_SHARD], bf16,
                                        isOutput=True)

    with tile.TileContext(nc) as tc:
        with (
            tc.tile_pool(name="const", bufs=1) as const,
            tc.tile_pool(name="work", bufs=1) as work,
            tc.tile_pool(name="psQKV", bufs=1, space="PSUM") as psQKV,
            tc.tile_pool(name="psT", bufs=2, space="PSUM") as psT,
            tc.tile_pool(name="psO", bufs=2, space="PSUM") as psO,
        ):
            # --- setup: stage every PE-read constant through DVE ---
            wqkv_dma = const.tile([F + 1, QKV_COLS], bf16, name="wqkv_dma")
            nc.sync.dma_start(out=wqkv_dma, in_=wqkv_ext[:])
            wqkv = const.tile([F + 1, QKV_COLS], bf16, name="wqkv")
            nc.vector.tensor_copy(wqkv[:], wqkv_dma[:])
            wo_dma = const.tile([F, F], bf16, name="wo_dma")
            nc.sync.dma_start(out=wo_dma, in_=wo_ext[:])
            wo_sb = const.tile([F, F], bf16, name="wo_sb")
            nc.vector.tensor_copy(wo_sb[:], wo_dma[:])
            id_dma = const.tile([P, P], bf16, name="id_dma")
            nc.sync.dma_start(out=id_dma, in_=id_ext[:])
            I128b = const.tile([P, P], bf16, name="I128b")
            nc.vector.tensor_copy(I128b[:], id_dma[:])
            bo_dma = const.tile([F, 1], f32, name="bo_dma")
            nc.sync.dma_start(out=bo_dma, in_=bo_ext[:].unsqueeze(1))
            bo_sb = const.tile([F, 1], f32, name="bo_sb")
            nc.vector.tensor_copy(bo_sb[:], bo_dma[:])

            act_warm = const.tile([1, 1], f32, name="act_warm")
            nc.scalar.activation(act_warm, bo_sb[0:1, 0:1],
                                 mybir.ActivationFunctionType.Exp)
            psDummy = psT.tile([1, 1], f32, tag="t", name="psDummy")
            nc.tensor.matmul(psDummy, lhsT=I128b[0:1, 0:1],
                             rhs=I128b[0:1, 0:1], start=True, stop=True)

            st = {}

            def stA_dma(m):  # x chunk for macro m
                xch = work.tile([F + 1, W * P], bf16, tag="xch", bufs=3,
                                name="xch")
                nc.sync.dma_start(
                    out=xch, in_=xt_ext[:, m * W * P:(m + 1) * W * P])
                st["x", m] = xch

            def stA_pe(m):  # 4 fused QKV projections
                xch = st["x", m]
                ps = psQKV.tile([P, W, 512], f32, tag="qkv", name="psBig")
                for w in range(W):
                    nc.tensor.matmul(ps[:, w, 0:QKV_COLS],
                                     lhsT=xch[:, w * P:(w + 1) * P],
                                     rhs=wqkv[:], start=True, stop=True)
                st["ps", m] = ps

            def stA_act(m):  # one fused PSUM -> SBUF copy
                ps = st.pop(("ps", m))
                st.pop(("x", m))
                qkv = work.tile([P, W, QKV_COLS], bf16, tag="qkv4", bufs=5,
                                name="qkv4")
                nc.scalar.activation(
                    qkv[:], ps[:, :, 0:QKV_COLS],
                    mybir.ActivationFunctionType.Copy)
                st["qkv", m] = qkv

            def stB_dve(m):  # QK products, per (subtile, head)
                qkv = st["qkv", m]
                prod = work.tile([P, W, H, S, S, HD], bf16, tag="prod",
                                 bufs=3, name="prod")
                for w in range(W):
                    qv = qkv[:, w, 0:F].rearrange(
                        "p (s h d) -> p s h d", s=S, h=H).transpose(
                        [0, 2, 1, 3])  # [p, h, q, d]
                    kv = qkv[:, w, F:2 * F].rearrange(
                        "p (s h d) -> p s h d", s=S, h=H).transpose(
                        [0, 2, 1, 3])
                    for h in range(H):
                        nc.vector.tensor_mul(
                            prod[:, w, h],
                            qv[:, h].unsqueeze(2).broadcast_to([P, S, S, HD]),
                            kv[:, h].unsqueeze(1).broadcast_to([P, S, S, HD]),
                        )
                st["prod", m] = prod

            def stC_pool(m):  # fused d-reduce
                prod = st.pop(("prod", m))
                dsum = work.tile([P, W, H, S, S], bf16, tag="dsum", bufs=2,
                                 name="dsum")
                nc.gpsimd.tensor_add(dsum[:], prod[:, :, :, :, :, 0],
                                     prod[:, :, :, :, :, 1])
                scores = work.tile([P, W, H, S, S], f32, tag="scores",
                                   bufs=3, name="scores")
                nc.gpsimd.tensor_add(scores[:], dsum[:],
                                     prod[:, :, :, :, :, 2])
                st["scores", m] = scores

            def stD_act(m):  # fused exp
                scores = st.pop(("scores", m))
                E = work.tile([P, W, H, S, S], bf16, tag="E", bufs=3,
                              name="E")
                nc.scalar.activation(E[:], scores[:],
                                     mybir.ActivationFunctionType.Exp)
                st["E", m] = E

            def stD_dve(m):  # denom + recip + AV products + tree t1, t2
                E = st.pop(("E", m))
                qkv = st.pop(("qkv", m))
                den = work.tile([P, W * H * S], f32, tag="den", bufs=2,
                                name="den")
                nc.vector.tensor_reduce(
                    den[:], E[:].rearrange("p w h q k -> p (w h q) k"),
                    axis=mybir.AxisListType.X, op=add)
                rden = work.tile([P, W * H * S], f32, tag="rden", bufs=4,
                                 name="rden")
                nc.vector.reciprocal_approx_fast(rden[:], den[:])
                st["rden", m] = rden

                p2 = work.tile([P, W, H, HD, S, S], bf16, tag="p2", bufs=2,
                               name="p2")
                for w in range(W):
                    vv = qkv[:, w, 2 * F:3 * F].rearrange(
                        "p (h d k) -> p h d k", h=H, d=HD)  # [p, h, d, k]
                    for h in range(H):
                        nc.vector.tensor_mul(
                            p2[:, w, h],
                            E[:, w, h].unsqueeze(1).broadcast_to(
                                [P, HD, S, S]),
                            vv[:, h].unsqueeze(2).broadcast_to(
                                [P, HD, S, S]),
                        )
                t1 = work.tile([P, W, H, HD, S, 8], bf16, tag="t1", bufs=2,
                               name="t1")
                nc.vector.tensor_add(t1[:], p2[:, :, :, :, :, 0:8],
                                     p2[:, :, :, :, :, 8:16])
                t2 = work.tile([P, W, H, HD, S, 4], bf16, tag="t2", bufs=2,
                               name="t2")
                nc.vector.tensor_add(t2[:], t1[:, :, :, :, :, 0:4],
                                     t1[:, :, :, :, :, 4:8])
                t3 = work.tile([P, W, H, HD, S, 2], bf16, tag="t3", bufs=2,
                               name="t3")
                nc.vector.tensor_add(t3[:], t2[:, :, :, :, :, 0:2],
                                     t2[:, :, :, :, :, 2:4])
                ctxd = work.tile([P, W, H, HD, S], f32, tag="ctxd", bufs=3,
                                 name="ctxd")
                nc.vector.tensor_add(ctxd[:], t3[:, :, :, :, :, 0],
                                     t3[:, :, :, :, :, 1])
                st["ctxd", m] = ctxd

            def stF_dve(m):  # fused normalize -> ctx in (h,d,q) order
                ctxd = st.pop(("ctxd", m))
                rden = st.pop(("rden", m))
                ctx = work.tile([P, W, H, HD, S], bf16, tag="ctx", bufs=2,
                                name="ctx")
                nc.vector.tensor_mul(
                    ctx[:],
                    ctxd[:],
                    rden[:].rearrange("p (w h q) -> p w h q", w=W, h=H)
                    .unsqueeze(3).broadcast_to([P, W, H, HD, S]),
                )
                st["ctx", m] = ctx

            def stG_tail(m):  # per-subtile: transpose, ctxT, outproj, store
                ctx = st.pop(("ctx", m))
                out4 = work.tile([F, W * P], bf16, tag="out4", bufs=3,
                                 name="out4")
                for w in range(W):
                    psC = psT.tile([F, P], bf16, tag="t", name="psC")
                    nc.tensor.transpose(
                        psC, ctx[:, w].rearrange("p h d q -> p (h d q)"),
                        I128b[:])
                    ctxT = work.tile([F, P], bf16, tag="ctxT", bufs=3,
                                     name="ctxT")
                    nc.scalar.activation(ctxT[:], psC[:],
                                         mybir.ActivationFunctionType.Copy)
                    psOutT = psO.tile([F, P], f32, tag="o", name="psOutT")
                    nc.tensor.matmul(psOutT, lhsT=wo_sb[:], rhs=ctxT[:],
                                     start=True, stop=True)
                    nc.scalar.activation(
                        out4[:, w * P:(w + 1) * P], psOutT[:],
                        mybir.ActivationFunctionType.Identity,
                        bias=bo_sb[:], scale=1.0)
                nc.sync.dma_start(
                    out=out_ext[:, m * W * P:(m + 1) * W * P], in_=out4[:])

            def live(j):
                return 0 <= j < NMACRO

            for i in range(NMACRO + 4):
                if live(i):
                    stA_dma(i)
                if live(i - 3):
                    stD_act(i - 3)       # ACT: exp first
                if live(i):
                    stA_pe(i)            # PE: projections
                if live(i):
                    stA_act(i)           # ACT: qkv copy
                if live(i - 1):
                    stB_dve(i - 1)       # DVE: QK products
                if live(i - 3):
                    stD_dve(i - 3)       # DVE: denom/recip/AV/tree
                if live(i - 2):
                    stC_pool(i - 2)      # Pool: d-reduce
                if live(i - 4):
                    stF_dve(i - 4)       # DVE: normalize
                if live(i - 4):
                    stG_tail(i - 4)      # PE/ACT tail + store

    return nc


def get_graph():
    if "nc" not in _nc_cache:
        nc = _build_graph()
        nc.finalize()
        _nc_cache["nc"] = nc
    return _nc_cache["nc"]


def prepare_in_maps(x, Wq, bq, Wk, bk, Wv, bv, Wo, bo):
    bf16 = ml_dtypes.bfloat16
    sc = 1.0 / np.sqrt(np.float32(HD))
    seg = np.asarray(SEG)
    bqf = (bq[seg].reshape(F) * sc).astype(np.float32)
    bkf = bk[seg].reshape(F).astype(np.float32)
    bvf = bv[seg].reshape(F).astype(np.float32)

    def aug(Wblk, bvec):
        return np.concatenate([Wblk, bvec[None, :]], axis=0)  # [97, 96]

    wq = aug(_build_weight(Wq, sc), bqf)        # cols (s, h, d)
    wk = aug(_build_weight(Wk), bkf)            # cols (s, h, d)
    wv_shd = aug(_build_weight(Wv), bvf)        # cols (s=k, h, d)
    # v cols -> (h, d, k) order
    perm = np.empty(F, np.int64)
    for h in range(H):
        for d in range(HD):
            for k in range(S):
                perm[h * HD * S + d * S + k] = k * D + h * HD + d
    wv = wv_shd[:, perm]
    wqkv = np.concatenate([wq, wk, wv], axis=1).astype(bf16)  # [97, 288]

    # wo rows permuted to (h, s, d) to match ctx row order; cols (s, o)
    wo_full = np.zeros((F, F), np.float32)
    for s in range(S):
        wo_full[s * D:(s + 1) * D, s * D:(s + 1) * D] = Wo.T
    rperm = np.empty(F, np.int64)
    for h in range(H):
        for d in range(HD):
            for s in range(S):
                rperm[h * S * HD + d * S + s] = s * D + h * HD + d
    wo = wo_full[rperm].astype(bf16)
    bof = np.tile(bo, S).astype(np.float32)
    ident = np.eye(P, dtype=bf16)

    xf = np.asarray(x, np.float32).reshape(B_TOTAL, F).astype(bf16)
    ones = np.ones((1, B_SHARD), dtype=bf16)
    in_maps = []
    for c in range(N_CORES):
        shard = np.concatenate([np.ascontiguousarray(
            xf[c * B_SHARD:(c + 1) * B_SHARD].T), ones], axis=0)  # [97, B]
        in_maps.append({"xt": shard, "wqkv": wqkv, "wo": wo,
                        "bo": bof, "ident": ident})
    return in_maps


def kernel(x, Wq, bq, Wk, bk, Wv, bv, Wo, bo):
    nc = get_graph()
    in_maps = prepare_in_maps(x, Wq, bq, Wk, bk, Wv, bv, Wo, bo)
    res = run_bass_kernel_spmd(nc, in_maps, core_ids=list(range(N_CORES)))
    outs = [np.asarray(res.results[c]["out"]).astype(np.float32).T
            for c in range(N_CORES)]  # each [32768, 96]
    out = np.concatenate(outs, axis=0)
    return np.ascontiguousarray(out.reshape(B_TOTAL, S, D))


# revision 3
# speedup vs baseline: 1.1018x; 1.0922x over previous
"""Trainium2 Bass kernel for nn_Attention_17454747091547 — v7.

Measured-HW-cost driven redesign. Per-instruction DVE overhead is ~500ns
on hardware, so everything fusable is fused across W=4 batch sub-tiles
(512 batch rows per macro-iteration); only the broadcast products
(QK, AV) stay per-(subtile, head) — their APs hit the 3-free-dim ISA
limit otherwise.

Per macro-iter (4 tiles, 512 batch):
  PE  : 4 fused QKV projections (one [97,288] matmul each into a
        bank-aligned PSUM slot), 4 ctx transposes, 4 out-projections
  ACT : 1 fused QKV PSUM->SBUF copy, 1 fused exp, 4 ctxT copies,
        4 bias+store copies
  DVE : 8 QK product muls, 1 denom tensor_reduce over E, 1 reciprocal,
        8 AV product muls, tree steps t1+t2, 1 fused normalize
  Pool: d-reduce (2 fused strided adds), tree steps t3+t4

x and output stream through small rotating SBUF buffers (one DMA per
macro-iter each way); weights/identity stay resident.
"""

import numpy as np
import ml_dtypes

import concourse.bass as bass
import concourse.tile as tile
from concourse import bacc
from concourse import mybir
from concourse.bass_utils import run_bass_kernel_spmd

SEG = [0, 1, 1, 1, 1, 1, 2, 2, 2, 3, 4, 4, 4, 4, 4, 4]
N_CORES = 8
B_TOTAL = 262144
B_SHARD = B_TOTAL // N_CORES  # 32768
P = 128
W = 4                      # sub-tiles per macro-iteration
NTILES = B_SHARD // P      # 256
NMACRO = NTILES // W       # 64
S = 16
D = 6
H = 2
HD = 3
F = S * D                  # 96
QKV_COLS = 3 * F           # 288 = q(96,(s,h,d)) | k(96,(s,h,d)) | v(96,(h,d,k))

_nc_cache = {}


def _build_weight(W_, scale=1.0):
    """[5,6,6] -> [96,96] f32 blockdiag of W[seg[s]].T (cols (s,o))."""
    Wa = np.zeros((F, F), np.float32)
    for s in range(S):
        Wa[s * D:(s + 1) * D, s * D:(s + 1) * D] = W_[SEG[s]].T * scale
    return Wa


def _build_graph():
    nc = bacc.Bacc()
    f32 = mybir.dt.float32
    bf16 = mybir.dt.bfloat16
    add = mybir.AluOpType.add

    xt_ext = nc.declare_dram_parameter("xt", [F + 1, B_SHARD], bf16,
                                       isOutput=False)
    wqkv_ext = nc.declare_dram_parameter("wqkv", [F + 1, QKV_COLS], bf16,
                                         isOutput=False)
    wo_ext = nc.declare_dram_parameter("wo", [F, F], bf16, isOutput=False)
    bo_ext = nc.declare_dram_parameter("bo", [F], f32, isOutput=False)
    id_ext = nc.declare_dram_parameter("ident", [P, P], bf16, isOutput=False)
    out_ext = nc.declare_dram_parameter("out", [F, B_SHARD], bf16,
                                        isOutput=True)

    with tile.TileContext(nc) as tc:
        with (
            tc.tile_pool(name="const", bufs=1) as const,
            tc.tile_pool(name="work", bufs=1) as work,
            tc.tile_pool(name="psQKV", bufs=1, space="PSUM") as psQKV,
            tc.tile_pool(name="psT", bufs=2, space="PSUM") as psT,
            tc.tile_pool(name="psO", bufs=2, space="PSUM") as psO,
        ):
            # --- setup: stage every PE-read constant through DVE ---
            wqkv_dma = const.tile([F + 1, QKV_COLS], bf16, name="wqkv_dma")
            nc.sync.dma_start(out=wqkv_dma, in_=wqkv_ext[:])
            wqkv = const.tile([F + 1, QKV_COLS], bf16, name="wqkv")
            nc.vector.tensor_copy(wqkv[:], wqkv_dma[:])
            wo_dma = const.tile([F, F], bf16, name="wo_dma")
            nc.sync.dma_start(out=wo_dma, in_=wo_ext[:])
            wo_sb = const.tile([F, F], bf16, name="wo_sb")
            nc.vector.tensor_copy(wo_sb[:], wo_dma[:])
            id_dma = const.tile([P, P], bf16, name="id_dma")
            nc.sync.dma_start(out=id_dma, in_=id_ext[:])
            I128b = const.tile([P, P], bf16, name="I128b")
            nc.vector.tensor_copy(I128b[:], id_dma[:])
            bo_dma = const.tile([F, 1], f32, name="bo_dma")
            nc.sync.dma_start(out=bo_dma, in_=bo_ext[:].unsqueeze(1))
            bo_sb = const.tile([F, 1], f32, name="bo_sb")
            nc.vector.tensor_copy(bo_sb[:], bo_dma[:])

            act_warm = const.tile([1, 1], f32, name="act_warm")
            nc.scalar.activation(act_warm, bo_sb[0:1, 0:1],
                                 mybir.ActivationFunctionType.Exp)
            psDummy = psT.tile([1, 1], f32, tag="t", name="psDummy")
            nc.tensor.matmul(psDummy, lhsT=I128b[0:1, 0:1],
                             rhs=I128b[0:1, 0:1], start=True, stop=True)

            st = {}

            def stA_dma(m):  # x chunk for macro m
                xch = work.tile([F + 1, W * P], bf16, tag="xch", bufs=3,
                                name="xch")
                nc.sync.dma_start(
                    out=xch, in_=xt_ext[:, m * W * P:(m + 1) * W * P])
                st["x", m] = xch

            def stA_pe(m):  # 4 fused QKV projections
                xch = st["x", m]
                ps = psQKV.tile([P, W, 512], f32, tag="qkv", name="psBig")
                for w in range(W):
                    nc.tensor.matmul(ps[:, w, 0:QKV_COLS],
                                     lhsT=xch[:, w * P:(w + 1) * P],
                                     rhs=wqkv[:], start=True, stop=True)
                st["ps", m] = ps

            def stA_act(m):  # PSUM -> SBUF copies: qk fused, v dense
                ps = st.pop(("ps", m))
                st.pop(("x", m))
                qkv = work.tile([P, W, 2 * F], bf16, tag="qkv4", bufs=5,
                                name="qkv4")
                nc.scalar.activation(
                    qkv[:], ps[:, :, 0:2 * F],
                    mybir.ActivationFunctionType.Copy)
                v4 = work.tile([P, W, H, HD, S], bf16, tag="v4", bufs=5,
                               name="v4")
                nc.scalar.activation(
                    v4[:].rearrange("p w h d k -> p w (h d k)"),
                    ps[:, :, 2 * F:3 * F],
                    mybir.ActivationFunctionType.Copy)
                st["qkv", m] = qkv
                st["v4", m] = v4

            def stB_dve(m):  # QK products, per (subtile, head)
                qkv = st["qkv", m]
                prod = work.tile([P, W, H, S, S, HD], bf16, tag="prod",
                                 bufs=3, name="prod")
                for w in range(W):
                    qv = qkv[:, w, 0:F].rearrange(
                        "p (s h d) -> p s h d", s=S, h=H).transpose(
                        [0, 2, 1, 3])  # [p, h, q, d]
                    kv = qkv[:, w, F:2 * F].rearrange(
                        "p (s h d) -> p s h d", s=S, h=H).transpose(
                        [0, 2, 1, 3])
                    for h in range(H):
                        nc.vector.tensor_mul(
                            prod[:, w, h],
                            qv[:, h].unsqueeze(2).broadcast_to([P, S, S, HD]),
                            kv[:, h].unsqueeze(1).broadcast_to([P, S, S, HD]),
                        )
                st["prod", m] = prod

            def stC_pool(m):  # fused d-reduce
                prod = st.pop(("prod", m))
                dsum = work.tile([P, W, H, S, S], bf16, tag="dsum", bufs=2,
                                 name="dsum")
                nc.gpsimd.tensor_add(dsum[:], prod[:, :, :, :, :, 0],
                                     prod[:, :, :, :, :, 1])
                scores = work.tile([P, W, H, S, S], f32, tag="scores",
                                   bufs=3, name="scores")
                nc.gpsimd.tensor_add(scores[:], dsum[:],
                                     prod[:, :, :, :, :, 2])
                st["scores", m] = scores

            def stD_act(m):  # fused exp
                scores = st.pop(("scores", m))
                E = work.tile([P, W, H, S, S], bf16, tag="E", bufs=3,
                              name="E")
                nc.scalar.activation(E[:], scores[:],
                                     mybir.ActivationFunctionType.Exp)
                st["E", m] = E

            def stD_dve(m):  # AV products, then denom + recip, tree
                E = st.pop(("E", m))
                st.pop(("qkv", m))
                v4 = st.pop(("v4", m))
                p2 = work.tile([P, W, H, HD, S, S], bf16, tag="p2", bufs=2,
                               name="p2")
                for w in range(W):
                    for h in range(H):
                        nc.vector.tensor_mul(
                            p2[:, w, h],
                            E[:, w, h].unsqueeze(1).broadcast_to(
                                [P, HD, S, S]),
                            v4[:, w, h].unsqueeze(2).broadcast_to(
                                [P, HD, S, S]),
                        )
                den = work.tile([P, W * H * S], f32, tag="den", bufs=2,
                                name="den")
                nc.vector.tensor_reduce(
                    den[:], E[:].rearrange("p w h q k -> p (w h q) k"),
                    axis=mybir.AxisListType.X, op=add)
                rden = work.tile([P, W * H * S], f32, tag="rden", bufs=4,
                                 name="rden")
                nc.vector.reciprocal_approx_fast(rden[:], den[:])
                st["rden", m] = rden
                t1 = work.tile([P, W, H, HD, S, 8], bf16, tag="t1", bufs=2,
                               name="t1")
                nc.vector.tensor_add(t1[:], p2[:, :, :, :, :, 0:8],
                                     p2[:, :, :, :, :, 8:16])
                t2 = work.tile([P, W, H, HD, S, 4], bf16, tag="t2", bufs=2,
                               name="t2")
                nc.vector.tensor_add(t2[:], t1[:, :, :, :, :, 0:4],
                                     t1[:, :, :, :, :, 4:8])
                t3 = work.tile([P, W, H, HD, S, 2], bf16, tag="t3", bufs=2,
                               name="t3")
                nc.vector.tensor_add(t3[:], t2[:, :, :, :, :, 0:2],
                                     t2[:, :, :, :, :, 2:4])
                ctxd = work.tile([P, W, H, HD, S], f32, tag="ctxd", bufs=3,
                                 name="ctxd")
                nc.vector.tensor_add(ctxd[:], t3[:, :, :, :, :, 0],
                                     t3[:, :, :, :, :, 1])
                st["ctxd", m] = ctxd

            def stF_dve(m):  # fused normalize -> ctx in (h,d,q) order
                ctxd = st.pop(("ctxd", m))
                rden = st.pop(("rden", m))
                ctx = work.tile([P, W, H, HD, S], bf16, tag="ctx", bufs=2,
                                name="ctx")
                nc.vector.tensor_mul(
                    ctx[:],
                    ctxd[:],
                    rden[:].rearrange("p (w h q) -> p w h q", w=W, h=H)
                    .unsqueeze(3).broadcast_to([P, W, H, HD, S]),
                )
                st["ctx", m] = ctx

            def stG_tail(m):  # per-subtile: transpose, ctxT, outproj, store
                ctx = st.pop(("ctx", m))
                out4 = work.tile([F, W * P], bf16, tag="out4", bufs=3,
                                 name="out4")
                for w in range(W):
                    psC = psT.tile([F, P], bf16, tag="t", name="psC")
                    nc.tensor.transpose(
                        psC, ctx[:, w].rearrange("p h d q -> p (h d q)"),
                        I128b[:])
                    ctxT = work.tile([F, P], bf16, tag="ctxT", bufs=3,
                                     name="ctxT")
                    nc.scalar.activation(ctxT[:], psC[:],
                                         mybir.ActivationFunctionType.Copy)
                    psOutT = psO.tile([F, P], f32, tag="o", name="psOutT")
                    nc.tensor.matmul(psOutT, lhsT=wo_sb[:], rhs=ctxT[:],
                                     start=True, stop=True)
                    nc.scalar.activation(
                        out4[:, w * P:(w + 1) * P], psOutT[:],
                        mybir.ActivationFunctionType.Identity,
                        bias=bo_sb[:], scale=1.0)
                nc.sync.dma_start(
                    out=out_ext[:, m * W * P:(m + 1) * W * P], in_=out4[:])

            def live(j):
                return 0 <= j < NMACRO

            for i in range(NMACRO + 4):
                if live(i):
                    stA_dma(i)
                if live(i - 3):
                    stD_act(i - 3)       # ACT: exp first
                if live(i):
                    stA_pe(i)            # PE: projections
                if live(i):
                    stA_act(i)           # ACT: qkv copy
                if live(i - 1):
                    stB_dve(i - 1)       # DVE: QK products
                if live(i - 3):
                    stD_dve(i - 3)       # DVE: denom/recip/AV/tree
                if live(i - 2):
                    stC_pool(i - 2)      # Pool: d-reduce
                if live(i - 4):
                    stF_dve(i - 4)       # DVE: normalize
                if live(i - 4):
                    stG_tail(i - 4)      # PE/ACT tail + store

    return nc


def get_graph():
    if "nc" not in _nc_cache:
        nc = _build_graph()
        nc.finalize()
        _nc_cache["nc"] = nc
    return _nc_cache["nc"]


def prepare_in_maps(x, Wq, bq, Wk, bk, Wv, bv, Wo, bo):
    bf16 = ml_dtypes.bfloat16
    sc = 1.0 / np.sqrt(np.float32(HD))
    seg = np.asarray(SEG)
    bqf = (bq[seg].reshape(F) * sc).astype(np.float32)
    bkf = bk[seg].reshape(F).astype(np.float32)
    bvf = bv[seg].reshape(F).astype(np.float32)

    def aug(Wblk, bvec):
        return np.concatenate([Wblk, bvec[None, :]], axis=0)  # [97, 96]

    wq = aug(_build_weight(Wq, sc), bqf)        # cols (s, h, d)
    wk = aug(_build_weight(Wk), bkf)            # cols (s, h, d)
    wv_shd = aug(_build_weight(Wv), bvf)        # cols (s=k, h, d)
    # v cols -> (h, d, k) order
    perm = np.empty(F, np.int64)
    for h in range(H):
        for d in range(HD):
            for k in range(S):
                perm[h * HD * S + d * S + k] = k * D + h * HD + d
    wv = wv_shd[:, perm]
    wqkv = np.concatenate([wq, wk, wv], axis=1).astype(bf16)  # [97, 288]

    # wo rows permuted to (h, s, d) to match ctx row order; cols (s, o)
    wo_full = np.zeros((F, F), np.float32)
    for s in range(S):
        wo_full[s * D:(s + 1) * D, s * D:(s + 1) * D] = Wo.T
    rperm = np.empty(F, np.int64)
    for h in range(H):
        for d in range(HD):
            for s in range(S):
                rperm[h * S * HD + d * S + s] = s * D + h * HD + d
    wo = wo_full[rperm].astype(bf16)
    bof = np.tile(bo, S).astype(np.float32)
    ident = np.eye(P, dtype=bf16)

    xf = np.asarray(x, np.float32).reshape(B_TOTAL, F).astype(bf16)
    ones = np.ones((1, B_SHARD), dtype=bf16)
    in_maps = []
    for c in range(N_CORES):
        shard = np.concatenate([np.ascontiguousarray(
            xf[c * B_SHARD:(c + 1) * B_SHARD].T), ones], axis=0)  # [97, B]
        in_maps.append({"xt": shard, "wqkv": wqkv, "wo": wo,
                        "bo": bof, "ident": ident})
    return in_maps


def kernel(x, Wq, bq, Wk, bk, Wv, bv, Wo, bo):
    nc = get_graph()
    in_maps = prepare_in_maps(x, Wq, bq, Wk, bk, Wv, bv, Wo, bo)
    res = run_bass_kernel_spmd(nc, in_maps, core_ids=list(range(N_CORES)))
    outs = [np.asarray(res.results[c]["out"]).astype(np.float32).T
            for c in range(N_CORES)]  # each [32768, 96]
    out = np.concatenate(outs, axis=0)
    return np.ascontiguousarray(out.reshape(B_TOTAL, S, D))


# revision 4
# speedup vs baseline: 1.1068x; 1.0045x over previous
"""Trainium2 Bass kernel for nn_Attention_17454747091547 — v11.

Measured-HW-cost driven redesign. Per-instruction DVE overhead is ~500ns
on hardware, so everything fusable is fused across W=4 batch sub-tiles
(512 batch rows per macro-iteration); only the broadcast products
(QK, AV) stay per-(subtile, head) — their APs hit the 3-free-dim ISA
limit otherwise.

Per macro-iter (4 tiles, 512 batch):
  PE  : 4 fused QKV projections (one [97,288] matmul each into a
        bank-aligned PSUM slot), 4 ctx transposes, 4 out-projections
  ACT : 1 fused QKV PSUM->SBUF copy, 1 fused exp, 4 ctxT copies,
        4 bias+store copies
  DVE : 8 QK product muls, 1 denom tensor_reduce over E, 1 reciprocal,
        8 AV product muls, tree steps t1+t2, 1 fused normalize
  Pool: d-reduce (2 fused strided adds), tree steps t3+t4

x and output stream through small rotating SBUF buffers (one DMA per
macro-iter each way); weights/identity stay resident.
"""

import numpy as np
import ml_dtypes

import concourse.bass as bass
import concourse.tile as tile
from concourse import bacc
from concourse import mybir
from concourse.bass_utils import run_bass_kernel_spmd

SEG = [0, 1, 1, 1, 1, 1, 2, 2, 2, 3, 4, 4, 4, 4, 4, 4]
N_CORES = 8
B_TOTAL = 262144
B_SHARD = B_TOTAL // N_CORES  # 32768
P = 128
W = 4                      # sub-tiles per macro-iteration
NTILES = B_SHARD // P      # 256
NMACRO = NTILES // W       # 64
S = 16
D = 6
H = 2
HD = 3
F = S * D                  # 96
QKV_COLS = 3 * F           # 288 = q(96,(s,h,d)) | k(96,(s,h,d)) | v(96,(h,d,k))

_nc_cache = {}


def _build_weight(W_, scale=1.0):
    """[5,6,6] -> [96,96] f32 blockdiag of W[seg[s]].T (cols (s,o))."""
    Wa = np.zeros((F, F), np.float32)
    for s in range(S):
        Wa[s * D:(s + 1) * D, s * D:(s + 1) * D] = W_[SEG[s]].T * scale
    return Wa


def _build_graph():
    nc = bacc.Bacc()
    f32 = mybir.dt.float32
    bf16 = mybir.dt.bfloat16
    add = mybir.AluOpType.add

    xt_ext = nc.declare_dram_parameter("xt", [F + 1, B_SHARD], bf16,
                                       isOutput=False)
    wqkv_ext = nc.declare_dram_parameter("wqkv", [F + 1, QKV_COLS], bf16,
                                         isOutput=False)
    wo_ext = nc.declare_dram_parameter("wo", [F, F], bf16, isOutput=False)
    bo_ext = nc.declare_dram_parameter("bo", [F], f32, isOutput=False)
    id_ext = nc.declare_dram_parameter("ident", [P, P], bf16, isOutput=False)
    out_ext = nc.declare_dram_parameter("out", [F, B_SHARD], bf16,
                                        isOutput=True)

    with tile.TileContext(nc) as tc:
        with (
            tc.tile_pool(name="const", bufs=1) as const,
            tc.tile_pool(name="work", bufs=1) as work,
            tc.tile_pool(name="psQKV", bufs=1, space="PSUM") as psQKV,
            tc.tile_pool(name="psT", bufs=2, space="PSUM") as psT,
            tc.tile_pool(name="psO", bufs=2, space="PSUM") as psO,
        ):
            # --- setup: stage every PE-read constant through DVE ---
            wqkv_dma = const.tile([F + 1, QKV_COLS], bf16, name="wqkv_dma")
            nc.sync.dma_start(out=wqkv_dma, in_=wqkv_ext[:])
            wqkv = const.tile([F + 1, QKV_COLS], bf16, name="wqkv")
            nc.vector.tensor_copy(wqkv[:], wqkv_dma[:])
            wo_dma = const.tile([F, F], bf16, name="wo_dma")
            nc.sync.dma_start(out=wo_dma, in_=wo_ext[:])
            wo_sb = const.tile([F, F], bf16, name="wo_sb")
            nc.vector.tensor_copy(wo_sb[:], wo_dma[:])
            id_dma = const.tile([P, P], bf16, name="id_dma")
            nc.sync.dma_start(out=id_dma, in_=id_ext[:])
            I128b = const.tile([P, P], bf16, name="I128b")
            nc.vector.tensor_copy(I128b[:], id_dma[:])
            bo_dma = const.tile([F, 1], f32, name="bo_dma")
            nc.sync.dma_start(out=bo_dma, in_=bo_ext[:].unsqueeze(1))
            bo_sb = const.tile([F, 1], f32, name="bo_sb")
            nc.vector.tensor_copy(bo_sb[:], bo_dma[:])

            act_warm = const.tile([1, 1], f32, name="act_warm")
            nc.scalar.activation(act_warm, bo_sb[0:1, 0:1],
                                 mybir.ActivationFunctionType.Exp)
            psDummy = psT.tile([1, 1], f32, tag="t", name="psDummy")
            nc.tensor.matmul(psDummy, lhsT=I128b[0:1, 0:1],
                             rhs=I128b[0:1, 0:1], start=True, stop=True)

            st = {}

            def stA_dma(m):  # x chunk for macro m
                xch = work.tile([F + 1, W * P], bf16, tag="xch", bufs=3,
                                name="xch")
                nc.sync.dma_start(
                    out=xch, in_=xt_ext[:, m * W * P:(m + 1) * W * P])
                st["x", m] = xch

            def stA_pe(m):  # 4 fused QKV projections
                xch = st["x", m]
                ps = psQKV.tile([P, W, 512], f32, tag="qkv", name="psBig")
                for w in range(W):
                    nc.tensor.matmul(ps[:, w, 0:QKV_COLS],
                                     lhsT=xch[:, w * P:(w + 1) * P],
                                     rhs=wqkv[:], start=True, stop=True)
                st["ps", m] = ps

            def stA_act(m):  # PSUM -> SBUF copies: qk fused, v dense
                ps = st.pop(("ps", m))
                st.pop(("x", m))
                qkv = work.tile([P, W, 2 * F], bf16, tag="qkv4", bufs=5,
                                name="qkv4")
                nc.scalar.activation(
                    qkv[:], ps[:, :, 0:2 * F],
                    mybir.ActivationFunctionType.Copy)
                v4 = work.tile([P, W, H, HD, S], bf16, tag="v4", bufs=5,
                               name="v4")
                nc.scalar.activation(
                    v4[:].rearrange("p w h d k -> p w (h d k)"),
                    ps[:, :, 2 * F:3 * F],
                    mybir.ActivationFunctionType.Copy)
                st["qkv", m] = qkv
                st["v4", m] = v4

            def stB_dve(m):  # QK products, per (subtile, head)
                qkv = st["qkv", m]
                prod = work.tile([P, W, H, S, S, HD], bf16, tag="prod",
                                 bufs=3, name="prod")
                for w in range(W):
                    qv = qkv[:, w, 0:F].rearrange(
                        "p (s h d) -> p s h d", s=S, h=H).transpose(
                        [0, 2, 1, 3])  # [p, h, q, d]
                    kv = qkv[:, w, F:2 * F].rearrange(
                        "p (s h d) -> p s h d", s=S, h=H).transpose(
                        [0, 2, 1, 3])
                    for h in range(H):
                        nc.vector.tensor_mul(
                            prod[:, w, h],
                            qv[:, h].unsqueeze(2).broadcast_to([P, S, S, HD]),
                            kv[:, h].unsqueeze(1).broadcast_to([P, S, S, HD]),
                        )
                st["prod", m] = prod

            def stC_pool(m):  # fused d-reduce
                prod = st.pop(("prod", m))
                dsum = work.tile([P, W, H, S, S], bf16, tag="dsum", bufs=2,
                                 name="dsum")
                nc.gpsimd.tensor_add(dsum[:], prod[:, :, :, :, :, 0],
                                     prod[:, :, :, :, :, 1])
                scores = work.tile([P, W, H, S, S], bf16, tag="scores",
                                   bufs=3, name="scores")
                nc.gpsimd.tensor_add(scores[:], dsum[:],
                                     prod[:, :, :, :, :, 2])
                st["scores", m] = scores

            def stD_act(m):  # fused exp
                scores = st.pop(("scores", m))
                E = work.tile([P, W, H, S, S], bf16, tag="E", bufs=3,
                              name="E")
                nc.scalar.activation(E[:], scores[:],
                                     mybir.ActivationFunctionType.Exp)
                st["E", m] = E

            def stD_dve(m):  # AV products, then denom + recip, tree
                E = st.pop(("E", m))
                st.pop(("qkv", m))
                v4 = st.pop(("v4", m))
                p2 = work.tile([P, W, H, HD, S, S], bf16, tag="p2", bufs=2,
                               name="p2")
                for w in range(W):
                    for h in range(H):
                        nc.vector.tensor_mul(
                            p2[:, w, h],
                            E[:, w, h].unsqueeze(1).broadcast_to(
                                [P, HD, S, S]),
                            v4[:, w, h].unsqueeze(2).broadcast_to(
                                [P, HD, S, S]),
                        )
                den = work.tile([P, W * H * S], f32, tag="den", bufs=2,
                                name="den")
                nc.vector.tensor_reduce(
                    den[:], E[:].rearrange("p w h q k -> p (w h q) k"),
                    axis=mybir.AxisListType.X, op=add)
                rden = work.tile([P, W * H * S], f32, tag="rden", bufs=4,
                                 name="rden")
                nc.vector.reciprocal_approx_fast(rden[:], den[:])
                st["rden", m] = rden
                ctxd = work.tile([P, W, H, HD, S], f32, tag="ctxd", bufs=3,
                                 name="ctxd")
                nc.vector.tensor_reduce(ctxd[:], p2[:], 
                                        axis=mybir.AxisListType.X, op=add)
                st["ctxd", m] = ctxd

            def stF_dve(m):  # fused normalize -> ctx in (h,d,q) order
                ctxd = st.pop(("ctxd", m))
                rden = st.pop(("rden", m))
                ctx = work.tile([P, W, H, HD, S], bf16, tag="ctx", bufs=2,
                                name="ctx")
                nc.vector.tensor_mul(
                    ctx[:],
                    ctxd[:],
                    rden[:].rearrange("p (w h q) -> p w h q", w=W, h=H)
                    .unsqueeze(3).broadcast_to([P, W, H, HD, S]),
                )
                st["ctx", m] = ctx

            def stG_tail(m):  # per-subtile: transpose, ctxT, outproj, store
                ctx = st.pop(("ctx", m))
                out4 = work.tile([F, W * P], bf16, tag="out4", bufs=3,
                                 name="out4")
                for w in range(W):
                    psC = psT.tile([F, P], bf16, tag="t", name="psC")
                    nc.tensor.transpose(
                        psC, ctx[:, w].rearrange("p h d q -> p (h d q)"),
                        I128b[:])
                    ctxT = work.tile([F, P], bf16, tag="ctxT", bufs=3,
                                     name="ctxT")
                    nc.scalar.activation(ctxT[:], psC[:],
                                         mybir.ActivationFunctionType.Copy)
                    psOutT = psO.tile([F, P], f32, tag="o", name="psOutT")
                    nc.tensor.matmul(psOutT, lhsT=wo_sb[:], rhs=ctxT[:],
                                     start=True, stop=True)
                    nc.scalar.activation(
                        out4[:, w * P:(w + 1) * P], psOutT[:],
                        mybir.ActivationFunctionType.Identity,
                        bias=bo_sb[:], scale=1.0)
                nc.sync.dma_start(
                    out=out_ext[:, m * W * P:(m + 1) * W * P], in_=out4[:])

            def live(j):
                return 0 <= j < NMACRO

            for i in range(NMACRO + 4):
                if live(i):
                    stA_dma(i)
                if live(i - 3):
                    stD_act(i - 3)       # ACT: exp first
                if live(i):
                    stA_pe(i)            # PE: projections
                if live(i):
                    stA_act(i)           # ACT: qkv copy
                if live(i - 1):
                    stB_dve(i - 1)       # DVE: QK products
                if live(i - 3):
                    stD_dve(i - 3)       # DVE: denom/recip/AV/tree
                if live(i - 2):
                    stC_pool(i - 2)      # Pool: d-reduce
                if live(i - 4):
                    stF_dve(i - 4)       # DVE: normalize
                if live(i - 4):
                    stG_tail(i - 4)      # PE/ACT tail + store

    return nc


def get_graph():
    if "nc" not in _nc_cache:
        nc = _build_graph()
        nc.finalize()
        _nc_cache["nc"] = nc
    return _nc_cache["nc"]


def prepare_in_maps(x, Wq, bq, Wk, bk, Wv, bv, Wo, bo):
    bf16 = ml_dtypes.bfloat16
    sc = 1.0 / np.sqrt(np.float32(HD))
    seg = np.asarray(SEG)
    bqf = (bq[seg].reshape(F) * sc).astype(np.float32)
    bkf = bk[seg].reshape(F).astype(np.float32)
    bvf = bv[seg].reshape(F).astype(np.float32)

    def aug(Wblk, bvec):
        return np.concatenate([Wblk, bvec[None, :]], axis=0)  # [97, 96]

    wq = aug(_build_weight(Wq, sc), bqf)        # cols (s, h, d)
    wk = aug(_build_weight(Wk), bkf)            # cols (s, h, d)
    wv_shd = aug(_build_weight(Wv), bvf)        # cols (s=k, h, d)
    # v cols -> (h, d, k) order
    perm = np.empty(F, np.int64)
    for h in range(H):
        for d in range(HD):
            for k in range(S):
                perm[h * HD * S + d * S + k] = k * D + h * HD + d
    wv = wv_shd[:, perm]
    wqkv = np.concatenate([wq, wk, wv], axis=1).astype(bf16)  # [97, 288]

    # wo rows permuted to (h, s, d) to match ctx row order; cols (s, o)
    wo_full = np.zeros((F, F), np.float32)
    for s in range(S):
        wo_full[s * D:(s + 1) * D, s * D:(s + 1) * D] = Wo.T
    rperm = np.empty(F, np.int64)
    for h in range(H):
        for d in range(HD):
            for s in range(S):
                rperm[h * S * HD + d * S + s] = s * D + h * HD + d
    wo = wo_full[rperm].astype(bf16)
    bof = np.tile(bo, S).astype(np.float32)
    ident = np.eye(P, dtype=bf16)

    xf = np.asarray(x, np.float32).reshape(B_TOTAL, F).astype(bf16)
    ones = np.ones((1, B_SHARD), dtype=bf16)
    in_maps = []
    for c in range(N_CORES):
        shard = np.concatenate([np.ascontiguousarray(
            xf[c * B_SHARD:(c + 1) * B_SHARD].T), ones], axis=0)  # [97, B]
        in_maps.append({"xt": shard, "wqkv": wqkv, "wo": wo,
                        "bo": bof, "ident": ident})
    return in_maps


def kernel(x, Wq, bq, Wk, bk, Wv, bv, Wo, bo):
    nc = get_graph()
    in_maps = prepare_in_maps(x, Wq, bq, Wk, bk, Wv, bv, Wo, bo)
    res = run_bass_kernel_spmd(nc, in_maps, core_ids=list(range(N_CORES)))
    outs = [np.asarray(res.results[c]["out"]).astype(np.float32).T
            for c in range(N_CORES)]  # each [32768, 96]
    out = np.concatenate(outs, axis=0)
    return np.ascontiguousarray(out.reshape(B_TOTAL, S, D))


# revision 5
# speedup vs baseline: 1.1168x; 1.0090x over previous
"""Trainium2 Bass kernel for nn_Attention_17454747091547 — v15.

Measured-HW-cost driven redesign. Per-instruction DVE overhead is ~500ns
on hardware, so everything fusable is fused across W=4 batch sub-tiles
(512 batch rows per macro-iteration); only the broadcast products
(QK, AV) stay per-(subtile, head) — their APs hit the 3-free-dim ISA
limit otherwise.

Per macro-iter (4 tiles, 512 batch):
  PE  : 4 fused QKV projections (one [97,288] matmul each into a
        bank-aligned PSUM slot), 4 ctx transposes, 4 out-projections
  ACT : 1 fused QKV PSUM->SBUF copy, 1 fused exp, 4 ctxT copies,
        4 bias+store copies
  DVE : 8 QK product muls, 1 denom tensor_reduce over E, 1 reciprocal,
        8 AV product muls, tree steps t1+t2, 1 fused normalize
  Pool: d-reduce (2 fused strided adds), tree steps t3+t4

x and output stream through small rotating SBUF buffers (one DMA per
macro-iter each way); weights/identity stay resident.
"""

import numpy as np
import ml_dtypes

import concourse.bass as bass
import concourse.tile as tile
from concourse import bacc
from concourse import mybir
from concourse.bass_utils import run_bass_kernel_spmd

SEG = [0, 1, 1, 1, 1, 1, 2, 2, 2, 3, 4, 4, 4, 4, 4, 4]
N_CORES = 8
B_TOTAL = 262144
B_SHARD = B_TOTAL // N_CORES  # 32768
P = 128
W = 4                      # sub-tiles per macro-iteration
NTILES = B_SHARD // P      # 256
NMACRO = NTILES // W       # 64
S = 16
D = 6
H = 2
HD = 3
F = S * D                  # 96
QKV_COLS = 3 * F           # 288 = q(96,(s,h,d)) | k(96,(s,h,d)) | v(96,(h,d,k))

_nc_cache = {}


def _build_weight(W_, scale=1.0):
    """[5,6,6] -> [96,96] f32 blockdiag of W[seg[s]].T (cols (s,o))."""
    Wa = np.zeros((F, F), np.float32)
    for s in range(S):
        Wa[s * D:(s + 1) * D, s * D:(s + 1) * D] = W_[SEG[s]].T * scale
    return Wa


def _build_graph():
    nc = bacc.Bacc()
    f32 = mybir.dt.float32
    bf16 = mybir.dt.bfloat16
    add = mybir.AluOpType.add

    xt_ext = nc.declare_dram_parameter("xt", [F + 1, B_SHARD], bf16,
                                       isOutput=False)
    wqkv_ext = nc.declare_dram_parameter("wqkv", [F + 1, QKV_COLS], bf16,
                                         isOutput=False)
    wo_ext = nc.declare_dram_parameter("wo", [F, F], bf16, isOutput=False)
    bo_ext = nc.declare_dram_parameter("bo", [F], f32, isOutput=False)
    id_ext = nc.declare_dram_parameter("ident", [P, P], bf16, isOutput=False)
    out_ext = nc.declare_dram_parameter("out", [F, B_SHARD], bf16,
                                        isOutput=True)

    with tile.TileContext(nc) as tc:
        with (
            tc.tile_pool(name="const", bufs=1) as const,
            tc.tile_pool(name="work", bufs=1) as work,
            tc.tile_pool(name="psQKV", bufs=1, space="PSUM") as psQKV,
            tc.tile_pool(name="psT", bufs=2, space="PSUM") as psT,
            tc.tile_pool(name="psO", bufs=2, space="PSUM") as psO,
        ):
            # --- setup: stage every PE-read constant through DVE ---
            wqkv_dma = const.tile([F + 1, QKV_COLS], bf16, name="wqkv_dma")
            nc.sync.dma_start(out=wqkv_dma, in_=wqkv_ext[:])
            wqkv = const.tile([F + 1, QKV_COLS], bf16, name="wqkv")
            nc.vector.tensor_copy(wqkv[:], wqkv_dma[:])
            wo_dma = const.tile([F, F], bf16, name="wo_dma")
            nc.sync.dma_start(out=wo_dma, in_=wo_ext[:])
            wo_sb = const.tile([F, F], bf16, name="wo_sb")
            nc.vector.tensor_copy(wo_sb[:], wo_dma[:])
            id_dma = const.tile([P, P], bf16, name="id_dma")
            nc.sync.dma_start(out=id_dma, in_=id_ext[:])
            I128b = const.tile([P, P], bf16, name="I128b")
            nc.vector.tensor_copy(I128b[:], id_dma[:])
            bo_dma = const.tile([F, 1], f32, name="bo_dma")
            nc.sync.dma_start(out=bo_dma, in_=bo_ext[:].unsqueeze(1))
            bo_sb = const.tile([F, 1], f32, name="bo_sb")
            nc.vector.tensor_copy(bo_sb[:], bo_dma[:])

            act_warm = const.tile([1, 1], f32, name="act_warm")
            nc.scalar.activation(act_warm, bo_sb[0:1, 0:1],
                                 mybir.ActivationFunctionType.Exp)
            psDummy = psT.tile([1, 1], f32, tag="t", name="psDummy")
            nc.tensor.matmul(psDummy, lhsT=I128b[0:1, 0:1],
                             rhs=I128b[0:1, 0:1], start=True, stop=True)

            st = {}

            def stA_dma(m):  # x chunk for macro m
                xch = work.tile([F + 1, W * P], bf16, tag="xch", bufs=3,
                                name="xch")
                nc.sync.dma_start(
                    out=xch, in_=xt_ext[:, m * W * P:(m + 1) * W * P])
                st["x", m] = xch

            def stA_pe(m):  # 4 fused QKV projections
                xch = st["x", m]
                ps = psQKV.tile([P, W, 512], f32, tag="qkv", name="psBig")
                for w in range(W):
                    nc.tensor.matmul(ps[:, w, 0:QKV_COLS],
                                     lhsT=xch[:, w * P:(w + 1) * P],
                                     rhs=wqkv[:], start=True, stop=True)
                st["ps", m] = ps

            def stA_act(m):  # PSUM -> SBUF copies: qk fused, v dense
                ps = st.pop(("ps", m))
                st.pop(("x", m))
                qkv = work.tile([P, W, 2 * F], bf16, tag="qkv4", bufs=5,
                                name="qkv4")
                nc.scalar.activation(
                    qkv[:], ps[:, :, 0:2 * F],
                    mybir.ActivationFunctionType.Copy)
                v4 = work.tile([P, W, H, HD, S], bf16, tag="v4", bufs=5,
                               name="v4")
                nc.scalar.activation(
                    v4[:].rearrange("p w h d k -> p w (h d k)"),
                    ps[:, :, 2 * F:3 * F],
                    mybir.ActivationFunctionType.Copy)
                st["qkv", m] = qkv
                st["v4", m] = v4

            def stB_dve(m):  # QK products, per (subtile, head)
                qkv = st["qkv", m]
                prod = work.tile([P, W, H, S, S, HD], bf16, tag="prod",
                                 bufs=3, name="prod")
                for w in range(W):
                    qv = qkv[:, w, 0:F].rearrange(
                        "p (s h d) -> p s h d", s=S, h=H).transpose(
                        [0, 2, 1, 3])  # [p, h, q, d]
                    kv = qkv[:, w, F:2 * F].rearrange(
                        "p (s h d) -> p s h d", s=S, h=H).transpose(
                        [0, 2, 1, 3])
                    for h in range(H):
                        nc.vector.tensor_mul(
                            prod[:, w, h],
                            qv[:, h].unsqueeze(2).broadcast_to([P, S, S, HD]),
                            kv[:, h].unsqueeze(1).broadcast_to([P, S, S, HD]),
                        )
                st["prod", m] = prod

            def stC_pool(m):  # fused d-reduce
                prod = st.pop(("prod", m))
                dsum = work.tile([P, W, H, S, S], bf16, tag="dsum", bufs=2,
                                 name="dsum")
                nc.gpsimd.tensor_add(dsum[:], prod[:, :, :, :, :, 0],
                                     prod[:, :, :, :, :, 1])
                scores = work.tile([P, W, H, S, S], bf16, tag="scores",
                                   bufs=3, name="scores")
                nc.gpsimd.tensor_add(scores[:], dsum[:],
                                     prod[:, :, :, :, :, 2])
                st["scores", m] = scores

            def stD_act(m):  # fused exp -> 4th plane of the product tensor
                scores = st.pop(("scores", m))
                p2 = work.tile([P, W, H, HD + 1, S, S], bf16, tag="p2",
                               bufs=2, name="p2")
                nc.scalar.activation(p2[:, :, :, HD], scores[:],
                                     mybir.ActivationFunctionType.Exp)
                st["p2", m] = p2

            def stD_dve(m):  # AV products + one TR -> ctx and denominators
                p2 = st.pop(("p2", m))
                st.pop(("qkv", m))
                v4 = st.pop(("v4", m))
                for w in range(W):
                    for h in range(H):
                        nc.vector.tensor_mul(
                            p2[:, w, h, 0:HD],
                            p2[:, w, h, HD].unsqueeze(1).broadcast_to(
                                [P, HD, S, S]),
                            v4[:, w, h].unsqueeze(2).broadcast_to(
                                [P, HD, S, S]),
                        )
                ctxd = work.tile([P, W, H, HD + 1, S], f32, tag="ctxd",
                                 bufs=3, name="ctxd")
                nc.vector.tensor_reduce(ctxd[:], p2[:], 
                                        axis=mybir.AxisListType.X, op=add)
                rden = work.tile([P, W, H, S], f32, tag="rden", bufs=4,
                                 name="rden")
                nc.vector.reciprocal_approx_fast(
                    rden[:].rearrange("p w h q -> p (w h) q"),
                    ctxd[:, :, :, HD].rearrange("p w h q -> p (w h) q"))
                st["rden", m] = rden
                st["ctxd", m] = ctxd

            def stF_dve(m):  # fused normalize -> ctx in (h,d,q) order
                ctxd = st.pop(("ctxd", m))
                rden = st.pop(("rden", m))
                ctx = work.tile([P, W, H, HD, S], bf16, tag="ctx", bufs=2,
                                name="ctx")
                nc.vector.tensor_mul(
                    ctx[:],
                    ctxd[:, :, :, 0:HD],
                    rden[:].unsqueeze(3).broadcast_to([P, W, H, HD, S]),
                )
                st["ctx", m] = ctx

            def stG_tail(m):  # per-subtile: transpose, ctxT, outproj, store
                ctx = st.pop(("ctx", m))
                out4 = work.tile([F, W * P], bf16, tag="out4", bufs=3,
                                 name="out4")
                for w in range(W):
                    psC = psT.tile([F, P], bf16, tag="t", name="psC")
                    nc.tensor.transpose(
                        psC, ctx[:, w].rearrange("p h d q -> p (h d q)"),
                        I128b[:])
                    ctxT = work.tile([F, P], bf16, tag="ctxT", bufs=3,
                                     name="ctxT")
                    nc.scalar.activation(ctxT[:], psC[:],
                                         mybir.ActivationFunctionType.Copy)
                    psOutT = psO.tile([F, P], f32, tag="o", name="psOutT")
                    nc.tensor.matmul(psOutT, lhsT=wo_sb[:], rhs=ctxT[:],
                                     start=True, stop=True)
                    nc.scalar.activation(
                        out4[:, w * P:(w + 1) * P], psOutT[:],
                        mybir.ActivationFunctionType.Identity,
                        bias=bo_sb[:], scale=1.0)
                nc.sync.dma_start(
                    out=out_ext[:, m * W * P:(m + 1) * W * P], in_=out4[:])

            def live(j):
                return 0 <= j < NMACRO

            for i in range(NMACRO + 4):
                if live(i):
                    stA_dma(i)
                if live(i - 3):
                    stD_act(i - 3)       # ACT: exp first
                if live(i):
                    stA_pe(i)            # PE: projections
                if live(i):
                    stA_act(i)           # ACT: qkv copy
                if live(i - 1):
                    stB_dve(i - 1)       # DVE: QK products
                if live(i - 3):
                    stD_dve(i - 3)       # DVE: denom/recip/AV/tree
                if live(i - 2):
                    stC_pool(i - 2)      # Pool: d-reduce
                if live(i - 4):
                    stF_dve(i - 4)       # DVE: normalize
                if live(i - 4):
                    stG_tail(i - 4)      # PE/ACT tail + store

    return nc


def get_graph():
    if "nc" not in _nc_cache:
        nc = _build_graph()
        nc.finalize()
        _nc_cache["nc"] = nc
    return _nc_cache["nc"]


def prepare_in_maps(x, Wq, bq, Wk, bk, Wv, bv, Wo, bo):
    bf16 = ml_dtypes.bfloat16
    sc = 1.0 / np.sqrt(np.float32(HD))
    seg = np.asarray(SEG)
    bqf = (bq[seg].reshape(F) * sc).astype(np.float32)
    bkf = bk[seg].reshape(F).astype(np.float32)
    bvf = bv[seg].reshape(F).astype(np.float32)

    def aug(Wblk, bvec):
        return np.concatenate([Wblk, bvec[None, :]], axis=0)  # [97, 96]

    wq = aug(_build_weight(Wq, sc), bqf)        # cols (s, h, d)
    wk = aug(_build_weight(Wk), bkf)            # cols (s, h, d)
    wv_shd = aug(_build_weight(Wv), bvf)        # cols (s=k, h, d)
    # v cols -> (h, d, k) order
    perm = np.empty(F, np.int64)
    for h in range(H):
        for d in range(HD):
            for k in range(S):
                perm[h * HD * S + d * S + k] = k * D + h * HD + d
    wv = wv_shd[:, perm]
    wqkv = np.concatenate([wq, wk, wv], axis=1).astype(bf16)  # [97, 288]

    # wo rows permuted to (h, s, d) to match ctx row order; cols (s, o)
    wo_full = np.zeros((F, F), np.float32)
    for s in range(S):
        wo_full[s * D:(s + 1) * D, s * D:(s + 1) * D] = Wo.T
    rperm = np.empty(F, np.int64)
    for h in range(H):
        for d in range(HD):
            for s in range(S):
                rperm[h * S * HD + d * S + s] = s * D + h * HD + d
    wo = wo_full[rperm].astype(bf16)
    bof = np.tile(bo, S).astype(np.float32)
    ident = np.eye(P, dtype=bf16)

    xf = np.asarray(x, np.float32).reshape(B_TOTAL, F).astype(bf16)
    ones = np.ones((1, B_SHARD), dtype=bf16)
    in_maps = []
    for c in range(N_CORES):
        shard = np.concatenate([np.ascontiguousarray(
            xf[c * B_SHARD:(c + 1) * B_SHARD].T), ones], axis=0)  # [97, B]
        in_maps.append({"xt": shard, "wqkv": wqkv, "wo": wo,
                        "bo": bof, "ident": ident})
    return in_maps


def kernel(x, Wq, bq, Wk, bk, Wv, bv, Wo, bo):
    nc = get_graph()
    in_maps = prepare_in_maps(x, Wq, bq, Wk, bk, Wv, bv, Wo, bo)
    res = run_bass_kernel_spmd(nc, in_maps, core_ids=list(range(N_CORES)))
    outs = [np.asarray(res.results[c]["out"]).astype(np.float32).T
            for c in range(N_CORES)]  # each [32768, 96]
    out = np.concatenate(outs, axis=0)
    return np.ascontiguousarray(out.reshape(B_TOTAL, S, D))


# revision 6
# speedup vs baseline: 1.1171x; 1.0003x over previous
"""Trainium2 Bass kernel for nn_Attention_17454747091547 — v19.

Measured-HW-cost driven redesign. Per-instruction DVE overhead is ~500ns
on hardware, so everything fusable is fused across W=4 batch sub-tiles
(512 batch rows per macro-iteration); only the broadcast products
(QK, AV) stay per-(subtile, head) — their APs hit the 3-free-dim ISA
limit otherwise.

Per macro-iter (4 tiles, 512 batch):
  PE  : 4 fused QKV projections (one [97,288] matmul each into a
        bank-aligned PSUM slot), 4 ctx transposes, 4 out-projections
  ACT : 1 fused QKV PSUM->SBUF copy, 1 fused exp, 4 ctxT copies,
        4 bias+store copies
  DVE : 8 QK product muls, 1 denom tensor_reduce over E, 1 reciprocal,
        8 AV product muls, tree steps t1+t2, 1 fused normalize
  Pool: d-reduce (2 fused strided adds), tree steps t3+t4

x and output stream through small rotating SBUF buffers (one DMA per
macro-iter each way); weights/identity stay resident.
"""

import numpy as np
import ml_dtypes

import concourse.bass as bass
import concourse.tile as tile
from concourse import bacc
from concourse import mybir
from concourse.bass_utils import run_bass_kernel_spmd

SEG = [0, 1, 1, 1, 1, 1, 2, 2, 2, 3, 4, 4, 4, 4, 4, 4]
N_CORES = 8
B_TOTAL = 262144
B_SHARD = B_TOTAL // N_CORES  # 32768
P = 128
W = 4                      # sub-tiles per macro-iteration
NTILES = B_SHARD // P      # 256
NMACRO = NTILES // W       # 64
S = 16
D = 6
H = 2
HD = 3
F = S * D                  # 96
QKV_COLS = 3 * F           # 288 = q(96,(s,h,d)) | k(96,(s,h,d)) | v(96,(h,d,k))

_nc_cache = {}


def _build_weight(W_, scale=1.0):
    """[5,6,6] -> [96,96] f32 blockdiag of W[seg[s]].T (cols (s,o))."""
    Wa = np.zeros((F, F), np.float32)
    for s in range(S):
        Wa[s * D:(s + 1) * D, s * D:(s + 1) * D] = W_[SEG[s]].T * scale
    return Wa


def _build_graph():
    nc = bacc.Bacc()
    f32 = mybir.dt.float32
    bf16 = mybir.dt.bfloat16
    add = mybir.AluOpType.add

    xt_ext = nc.declare_dram_parameter("xt", [F + 1, B_SHARD], bf16,
                                       isOutput=False)
    wqkv_ext = nc.declare_dram_parameter("wqkv", [F + 1, QKV_COLS], bf16,
                                         isOutput=False)
    wo_ext = nc.declare_dram_parameter("wo", [F, F], bf16, isOutput=False)
    bo_ext = nc.declare_dram_parameter("bo", [F], f32, isOutput=False)
    id_ext = nc.declare_dram_parameter("ident", [P, P], bf16, isOutput=False)
    out_ext = nc.declare_dram_parameter("out", [F, B_SHARD], bf16,
                                        isOutput=True)

    with tile.TileContext(nc) as tc:
        with (
            tc.tile_pool(name="const", bufs=1) as const,
            tc.tile_pool(name="work", bufs=1) as work,
            tc.tile_pool(name="psQKV", bufs=1, space="PSUM") as psQKV,
            tc.tile_pool(name="psT", bufs=2, space="PSUM") as psT,
            tc.tile_pool(name="psO", bufs=2, space="PSUM") as psO,
        ):
            # --- setup: stage every PE-read constant through DVE ---
            wqkv_dma = const.tile([F + 1, QKV_COLS], bf16, name="wqkv_dma")
            nc.sync.dma_start(out=wqkv_dma, in_=wqkv_ext[:])
            wqkv = const.tile([F + 1, QKV_COLS], bf16, name="wqkv")
            nc.vector.tensor_copy(wqkv[:], wqkv_dma[:])
            wo_dma = const.tile([F, F], bf16, name="wo_dma")
            nc.sync.dma_start(out=wo_dma, in_=wo_ext[:])
            wo_sb = const.tile([F, F], bf16, name="wo_sb")
            nc.vector.tensor_copy(wo_sb[:], wo_dma[:])
            id_dma = const.tile([P, P], bf16, name="id_dma")
            nc.sync.dma_start(out=id_dma, in_=id_ext[:])
            I128b = const.tile([P, P], bf16, name="I128b")
            nc.vector.tensor_copy(I128b[:], id_dma[:])
            bo_dma = const.tile([F, 1], f32, name="bo_dma")
            nc.sync.dma_start(out=bo_dma, in_=bo_ext[:].unsqueeze(1))
            bo_sb = const.tile([F, 1], f32, name="bo_sb")
            nc.vector.tensor_copy(bo_sb[:], bo_dma[:])

            act_warm = const.tile([1, 1], f32, name="act_warm")
            nc.scalar.activation(act_warm, bo_sb[0:1, 0:1],
                                 mybir.ActivationFunctionType.Exp)
            act_warm2 = const.tile([1, 1], f32, name="act_warm2")
            nc.scalar.activation(act_warm2, act_warm[0:1, 0:1],
                                 mybir.ActivationFunctionType.Ln)
            psDummy = psT.tile([1, 1], f32, tag="t", name="psDummy")
            nc.tensor.matmul(psDummy, lhsT=I128b[0:1, 0:1],
                             rhs=I128b[0:1, 0:1], start=True, stop=True)

            st = {}

            def stA_dma(m):  # x chunk for macro m
                xch = work.tile([F + 1, W * P], bf16, tag="xch", bufs=3,
                                name="xch")
                nc.sync.dma_start(
                    out=xch, in_=xt_ext[:, m * W * P:(m + 1) * W * P])
                st["x", m] = xch

            def stA_pe(m):  # 4 fused QKV projections
                xch = st["x", m]
                ps = psQKV.tile([P, W, 512], f32, tag="qkv", name="psBig")
                for w in range(W):
                    nc.tensor.matmul(ps[:, w, 0:QKV_COLS],
                                     lhsT=xch[:, w * P:(w + 1) * P],
                                     rhs=wqkv[:], start=True, stop=True)
                st["ps", m] = ps

            def stA_act(m):  # PSUM -> SBUF copies: qk fused, v dense
                ps = st.pop(("ps", m))
                st.pop(("x", m))
                qkv = work.tile([P, W, 2 * F], bf16, tag="qkv4", bufs=5,
                                name="qkv4")
                nc.scalar.activation(
                    qkv[:], ps[:, :, 0:2 * F],
                    mybir.ActivationFunctionType.Copy)
                v4 = work.tile([P, W, H, HD, S], bf16, tag="v4", bufs=5,
                               name="v4")
                nc.scalar.activation(
                    v4[:].rearrange("p w h d k -> p w (h d k)"),
                    ps[:, :, 2 * F:3 * F],
                    mybir.ActivationFunctionType.Copy)
                st["qkv", m] = qkv
                st["v4", m] = v4

            def stB_dve(m):  # QK products, per (subtile, head)
                qkv = st["qkv", m]
                prod = work.tile([P, W, H, S, S, HD], bf16, tag="prod",
                                 bufs=3, name="prod")
                for w in range(W):
                    qv = qkv[:, w, 0:F].rearrange(
                        "p (s h d) -> p s h d", s=S, h=H).transpose(
                        [0, 2, 1, 3])  # [p, h, q, d]
                    kv = qkv[:, w, F:2 * F].rearrange(
                        "p (s h d) -> p s h d", s=S, h=H).transpose(
                        [0, 2, 1, 3])
                    for h in range(H):
                        nc.vector.tensor_mul(
                            prod[:, w, h],
                            qv[:, h].unsqueeze(2).broadcast_to([P, S, S, HD]),
                            kv[:, h].unsqueeze(1).broadcast_to([P, S, S, HD]),
                        )
                st["prod", m] = prod

            def stC_pool(m):  # fused d-reduce
                prod = st.pop(("prod", m))
                dsum = work.tile([P, W, H, S, S], bf16, tag="dsum", bufs=2,
                                 name="dsum")
                nc.gpsimd.tensor_add(dsum[:], prod[:, :, :, :, :, 0],
                                     prod[:, :, :, :, :, 1])
                scores = work.tile([P, W, H, S, S], bf16, tag="scores",
                                   bufs=3, name="scores")
                nc.gpsimd.tensor_add(scores[:], dsum[:],
                                     prod[:, :, :, :, :, 2])
                st["scores", m] = scores

            def stD_act(m):  # fused exp -> 4th plane of the product tensor
                scores = st.pop(("scores", m))
                p2 = work.tile([P, W, H, HD + 1, S, S], bf16, tag="p2",
                               bufs=2, name="p2")
                nc.scalar.activation(p2[:, :, :, HD], scores[:],
                                     mybir.ActivationFunctionType.Exp)
                st["p2", m] = p2

            def stD_dve(m):  # AV products + one TR -> ctx and denominators
                p2 = st.pop(("p2", m))
                st.pop(("qkv", m))
                v4 = st.pop(("v4", m))
                for w in range(W):
                    for h in range(H):
                        nc.vector.tensor_mul(
                            p2[:, w, h, 0:HD],
                            p2[:, w, h, HD].unsqueeze(1).broadcast_to(
                                [P, HD, S, S]),
                            v4[:, w, h].unsqueeze(2).broadcast_to(
                                [P, HD, S, S]),
                        )
                ctxd = work.tile([P, W, H, HD + 1, S], f32, tag="ctxd",
                                 bufs=3, name="ctxd")
                nc.vector.tensor_reduce(ctxd[:], p2[:], 
                                        axis=mybir.AxisListType.X, op=add)
                st["ctxd", m] = ctxd

            def stE_act(m):  # rden = exp(-ln(den)) on the scalar engine
                ctxd = st[("ctxd", m)]
                lnden = work.tile([P, W, H, S], f32, tag="lnden", bufs=2,
                                  name="lnden")
                nc.scalar.activation(
                    lnden[:].rearrange("p w h q -> p (w h) q"),
                    ctxd[:, :, :, HD].rearrange("p w h q -> p (w h) q"),
                    mybir.ActivationFunctionType.Ln)
                rden = work.tile([P, W, H, S], f32, tag="rden", bufs=3,
                                 name="rden")
                nc.scalar.activation(rden[:], lnden[:],
                                     mybir.ActivationFunctionType.Exp,
                                     scale=-1.0)
                st["rden", m] = rden

            def stF_dve(m):  # fused normalize -> ctx in (h,d,q) order
                ctxd = st.pop(("ctxd", m))
                rden = st.pop(("rden", m))
                ctx = work.tile([P, W, H, HD, S], bf16, tag="ctx", bufs=2,
                                name="ctx")
                nc.vector.tensor_mul(
                    ctx[:],
                    ctxd[:, :, :, 0:HD],
                    rden[:].unsqueeze(3).broadcast_to([P, W, H, HD, S]),
                )
                st["ctx", m] = ctx

            def stG_tail(m):  # per-subtile: transpose, ctxT, outproj, store
                ctx = st.pop(("ctx", m))
                out4 = work.tile([F, W * P], bf16, tag="out4", bufs=3,
                                 name="out4")
                for w in range(W):
                    psC = psT.tile([F, P], bf16, tag="t", name="psC")
                    nc.tensor.transpose(
                        psC, ctx[:, w].rearrange("p h d q -> p (h d q)"),
                        I128b[:])
                    ctxT = work.tile([F, P], bf16, tag="ctxT", bufs=3,
                                     name="ctxT")
                    nc.scalar.activation(ctxT[:], psC[:],
                                         mybir.ActivationFunctionType.Copy)
                    psOutT = psO.tile([F, P], f32, tag="o", name="psOutT")
                    nc.tensor.matmul(psOutT, lhsT=wo_sb[:], rhs=ctxT[:],
                                     start=True, stop=True)
                    nc.scalar.activation(
                        out4[:, w * P:(w + 1) * P], psOutT[:],
                        mybir.ActivationFunctionType.Identity,
                        bias=bo_sb[:], scale=1.0)
                nc.sync.dma_start(
                    out=out_ext[:, m * W * P:(m + 1) * W * P], in_=out4[:])

            def live(j):
                return 0 <= j < NMACRO

            for i in range(NMACRO + 4):
                if live(i):
                    stA_dma(i)
                if live(i - 4):
                    stE_act(i - 4)       # ACT: rden = exp(-ln(den))
                if live(i - 3):
                    stD_act(i - 3)       # ACT: exp first
                if live(i):
                    stA_pe(i)            # PE: projections
                if live(i):
                    stA_act(i)           # ACT: qkv copy
                if live(i - 1):
                    stB_dve(i - 1)       # DVE: QK products
                if live(i - 3):
                    stD_dve(i - 3)       # DVE: denom/recip/AV/tree
                if live(i - 2):
                    stC_pool(i - 2)      # Pool: d-reduce
                if live(i - 4):
                    stF_dve(i - 4)       # DVE: normalize
                if live(i - 4):
                    stG_tail(i - 4)      # PE/ACT tail + store

    return nc


def get_graph():
    if "nc" not in _nc_cache:
        nc = _build_graph()
        nc.finalize()
        _nc_cache["nc"] = nc
    return _nc_cache["nc"]


def prepare_in_maps(x, Wq, bq, Wk, bk, Wv, bv, Wo, bo):
    bf16 = ml_dtypes.bfloat16
    sc = 1.0 / np.sqrt(np.float32(HD))
    seg = np.asarray(SEG)
    bqf = (bq[seg].reshape(F) * sc).astype(np.float32)
    bkf = bk[seg].reshape(F).astype(np.float32)
    bvf = bv[seg].reshape(F).astype(np.float32)

    def aug(Wblk, bvec):
        return np.concatenate([Wblk, bvec[None, :]], axis=0)  # [97, 96]

    wq = aug(_build_weight(Wq, sc), bqf)        # cols (s, h, d)
    wk = aug(_build_weight(Wk), bkf)            # cols (s, h, d)
    wv_shd = aug(_build_weight(Wv), bvf)        # cols (s=k, h, d)
    # v cols -> (h, d, k) order
    perm = np.empty(F, np.int64)
    for h in range(H):
        for d in range(HD):
            for k in range(S):
                perm[h * HD * S + d * S + k] = k * D + h * HD + d
    wv = wv_shd[:, perm]
    wqkv = np.concatenate([wq, wk, wv], axis=1).astype(bf16)  # [97, 288]

    # wo rows permuted to (h, s, d) to match ctx row order; cols (s, o)
    wo_full = np.zeros((F, F), np.float32)
    for s in range(S):
        wo_full[s * D:(s + 1) * D, s * D:(s + 1) * D] = Wo.T
    rperm = np.empty(F, np.int64)
    for h in range(H):
        for d in range(HD):
            for s in range(S):
                rperm[h * S * HD + d * S + s] = s * D + h * HD + d
    wo = wo_full[rperm].astype(bf16)
    bof = np.tile(bo, S).astype(np.float32)
    ident = np.eye(P, dtype=bf16)

    xf = np.asarray(x, np.float32).reshape(B_TOTAL, F).astype(bf16)
    ones = np.ones((1, B_SHARD), dtype=bf16)
    in_maps = []
    for c in range(N_CORES):
        shard = np.concatenate([np.ascontiguousarray(
            xf[c * B_SHARD:(c + 1) * B_SHARD].T), ones], axis=0)  # [97, B]
        in_maps.append({"xt": shard, "wqkv": wqkv, "wo": wo,
                        "bo": bof, "ident": ident})
    return in_maps


def kernel(x, Wq, bq, Wk, bk, Wv, bv, Wo, bo):
    nc = get_graph()
    in_maps = prepare_in_maps(x, Wq, bq, Wk, bk, Wv, bv, Wo, bo)
    res = run_bass_kernel_spmd(nc, in_maps, core_ids=list(range(N_CORES)))
    outs = [np.asarray(res.results[c]["out"]).astype(np.float32).T
            for c in range(N_CORES)]  # each [32768, 96]
    out = np.concatenate(outs, axis=0)
    return np.ascontiguousarray(out.reshape(B_TOTAL, S, D))
